# revision 53
# baseline (speedup 1.0000x reference)
"""Causal single-head attention on 8 Trainium2 NeuronCores.  ~174 us HW
(baseline 210-219 us), rel err 1.45e-2 (gate 2e-2).

Problem: x[4, 2048, 1024] fp32, Wq/Wk/Wv[1024, 1024] fp32.
  q,k,v = x@Wq, x@Wk, x@Wv ; out = softmax(mask(q k^T)/32) @ v

Sharding (SPMD — one program, 8 cores, per-core data):
  core = 2*b + h  handles batch b, queries {t : t % 2 == h} (1024 queries).
  The interleaved (mod-2) query split makes the causal block structure
  shape-identical across cores: per-core q-block jb (256 queries, spanning
  global positions [512*jb, 512*jb+512)) needs k-tiles 0..4*(jb+1)-1 on
  every core.  Causal masking inside the 4 diagonal k-tiles depends only on
  (u = t-4*jb, parity h) => 4 mask tiles passed as per-core data.

Work split within a core pair (same batch):
  - K^T and V projections: each core projects only its own 1024-token
    half (from xTk); halves exchange with pair-AllGathers (K fp8, split
    per 512-token window; V bf16, 2 MB -> 4 MB) on the serial CC stream.
  - Q^T projection: own 1024 queries (disjoint across the pair).

Dtypes / PE modes (HW-measured rates):
  - Q^T/K^T stored fp8 (e4m3) in a DoubleRow two-slot layout
    ([P, DC/2, 2, cols]; slot s = d_out chunk 2g+s); score matmuls run
    MatmulPerfMode.DoubleRow = K=256 per instruction at 1 cycle/row
    (2x bf16 throughput on HW).  fp8 Q/K rounding floor ~1.4e-2 rel.
  - Projections bf16 (fp8 inputs there push rel err past 2e-2).
  - expS / V / context matmuls bf16 (fp8 V or expS cost ~3.5e-2).
    Output written bf16, upcast on host.
  - softmax denominator: DVE pairwise bf16 adds (2x rate) + fp32 chain,
    interleaved with score tiles; one tiny fp32 ones-matmul per q-sub
    reduces across partitions, emitted one q-block late so the PE never
    waits on the DVE chain.  Causal mask is multiplicative 0/1 bf16
    applied to expS after exp (exp never waits on the DVE).  The two
    fully-masked k-tiles per q-sub-0 are skipped in the context matmuls.

Schedule (the whole kernel is a DMA-deadline problem; three initiator
FIFOs: sync/scalar HWDGE ~95/75 GB/s, gpsimd SWDGE ~80 GB/s + 0.65 us
engine-blocking per descriptor):
  PE:    K-w0 | K-w1 | V | Q | all scores | all contexts
  CCs:   K-w0 gather, K-w1 gather, V gather (serial stream; ~18 us
         launch after staging semaphores, ~25 us one-time bootstrap)
  A dma trigger's semaphore wait blocks its whole engine stream, so the
  K unpack (sync) is emitted before the Q projection (it may only block
  K-dependent work) and the V unpack (sync+scalar, interleaved by tile
  parity) is emitted AFTER the scores so the scores never gate on the V
  gather.  ctx outputs ride scalar behind the V unpack with an 8-deep
  ring.
"""

import os
import numpy as np
import ml_dtypes

import concourse.mybir as mybir
import concourse.tile as tile
from concourse import bacc

F32 = mybir.dt.float32
F32R = mybir.dt.float32r
BF16 = mybir.dt.bfloat16
F8 = mybir.dt.float8e4
BF16_NP = ml_dtypes.bfloat16
F8_NP = ml_dtypes.float8_e4m3
DR = mybir.MatmulPerfMode.DoubleRow

B, T, D = 4, 2048, 1024
P = 128
DC = D // P          # 8 contraction chunks
NT = T // P          # 16 key tiles
QB = 256             # queries per q-block (per core)
NJB = (T // 2) // QB # 4 q-blocks per core
SCALE = 1.0 / 32.0   # 1/sqrt(D)
MASK_NEG = -1.0e9
PAIRS = [[0, 1], [2, 3], [4, 5], [6, 7]]
_EXP = mybir.ActivationFunctionType.Exp
OFF = [0, 4, 12, 24]  # expS tile offset per q-block (cumsum of kt)
NKT = 40              # sum of kt over q-blocks


def _emit(nc, tc, xTk_d, xTq_d, wq_d, wk_d, wv_d, masks_d, out_d):
    HT = T // 2  # queries per core

    def mm(out, lhsT, rhs, start, stop, **kw):
        nc.tensor.matmul(out, lhsT, rhs, start=start, stop=stop, **kw)

    with (
        tc.sbuf_pool(name="persist", bufs=1) as persist,
        tc.sbuf_pool(name="attnp", bufs=1) as attnp,
        tc.sbuf_pool(name="recipp", bufs=4) as recip_pool,
        tc.sbuf_pool(name="accp", bufs=2) as acc_pool,
        tc.sbuf_pool(name="pairp", bufs=3) as pair_pool,
        tc.sbuf_pool(name="outp", bufs=8) as out_pool,
        tc.psum_pool(name="p512", bufs=3) as p512,
        tc.psum_pool(name="p256", bufs=4) as p256,
        tc.psum_pool(name="pden", bufs=1) as pden,
        tc.tile_pool(name="drp", bufs=1, space="DRAM") as dr_pool,
    ):
        # persistent SBUF tensors.  K/Q are fp8 in the DoubleRow two-slot
        # layout: dim1 = chunk-pair g, dim2 = slot (d_out chunk 2g+slot).
        K_sb = persist.tile([P, DC // 2, 2, T], F8, tag="K", name="K_sb")
        V_sb = persist.tile([P, NT * D], BF16, tag="V", name="V_sb")
        Q_sb = persist.tile([P, DC // 2, 2, HT], F8, tag="Q", name="Q_sb")
        # multiplicative 0/1 causal mask in bf16 (applied to expS AFTER the
        # exp — DVE runs at 2x rate on bf16 and the exp never waits on DVE)
        mask_sb = persist.tile([P, 4 * QB], BF16, tag="M", name="mask_sb")
        ones_f32 = persist.tile([P, 1], F32, tag="O32", name="ones_f32")

        with (
            tc.sbuf_pool(name="wkvp", bufs=1) as wkv_pool,
            tc.sbuf_pool(name="xtwp", bufs=2) as xtw_pool,
            tc.sbuf_pool(name="stgp", bufs=6) as stg_pool,
            nc.named_scope("kv_proj"),
        ):
            wk_sb = wkv_pool.tile([P, DC * D], BF16, tag="wk", name="wk_sb")
            wv_sb = wkv_pool.tile([P, DC * D], BF16, tag="wv", name="wv_sb")
            klocs = [dr_pool.tile([D, 512], F8, tag=f"kloc{w}",
                                  name=f"kloc{w}") for w in range(2)]
            kgs = [dr_pool.tile([2, D, 512], F8, tag=f"kg{w}",
                                name=f"kg{w}") for w in range(2)]
            vloc = dr_pool.tile([1024, D], BF16, tag="vloc", name="vloc")
            vg = dr_pool.tile([2, 1024, D], BF16, tag="vg", name="vg")

            # The whole projection phase is DMA-feed-paced, so every
            # transfer sits on a queue that is idle when its consumer
            # needs it, in consumption order.  Three initiator queues
            # exist (sync + scalar HWDGE, ~75-95 GB/s each; gpsimd SWDGE
            # ~80 GB/s but ~0.7us engine-blocking per descriptor, and any
            # SWDGE emitted after a collective trigger waits for that
            # collective to COMPLETE — so gpsimd only carries xtq up
            # front and the ctx outputs at the end).  The K AllGather is
            # split per 512-token window with each trigger emitted
            # immediately after its staging writes (a trigger's semaphore
            # wait covers every earlier DMA on its queues).
            xtks = [xtw_pool.tile([P, DC * 512], BF16, tag="xtw",
                                  name="xtk", bufs=2) for _ in range(2)]
            # wk splits scalar/gpsimd (fully landed ~13us), xtk rides sync,
            # wv splits gpsimd/scalar behind wk (landed ~27us, just before
            # the V projection's first psum completes).
            for c in range(0, DC, 2):
                nc.scalar.dma_start(out=wk_sb[:, c * D:(c + 1) * D],
                                    in_=wk_d[c * P:(c + 1) * P, :])
            for c in range(1, DC, 2):
                nc.gpsimd.dma_start(out=wk_sb[:, c * D:(c + 1) * D],
                                    in_=wk_d[c * P:(c + 1) * P, :])
            for c in range(0, DC, 2):
                nc.gpsimd.dma_start(out=wv_sb[:, c * D:(c + 1) * D],
                                    in_=wv_d[c * P:(c + 1) * P, :])
            for c in range(1, DC, 2):
                nc.scalar.dma_start(out=wv_sb[:, c * D:(c + 1) * D],
                                    in_=wv_d[c * P:(c + 1) * P, :])
            for c in range(DC):
                nc.sync.dma_start(
                    out=xtks[0][:, c * 512:(c + 1) * 512],
                    in_=xTk_d[c * P:(c + 1) * P, 0:512])
            for c in range(DC):
                nc.sync.dma_start(
                    out=xtks[1][:, c * 512:(c + 1) * 512],
                    in_=xTk_d[c * P:(c + 1) * P, 512:1024])
            xtqs = []
            for jp in range(NJB // 2):
                xtq = xtw_pool.tile([P, DC * 512], BF16, tag="xtq",
                                    name="xtq", bufs=2)
                for c in range(DC):
                    nc.gpsimd.dma_start(
                        out=xtq[:, c * 512:(c + 1) * 512],
                        in_=xTq_d[c * P:(c + 1) * P, 512 * jp:512 * (jp + 1)])
                xtqs.append(xtq)

            def k_proj_window(w):
                xtk = xtks[w]
                for c2 in range(DC):
                    ps = p512.tile([P, 512], F32, tag="mm512", name="ps_k")
                    for c in range(DC):
                        mm(ps, wk_sb[:, c * D + P * c2: c * D + P * (c2 + 1)],
                           xtk[:, c * 512:(c + 1) * 512], c == 0, c == DC - 1)
                    st = stg_pool.tile([P, 512], F8, tag="stk", name="stk")
                    nc.scalar.copy(out=st, in_=ps)
                    nc.sync.dma_start(out=klocs[w][c2 * P:(c2 + 1) * P, :],
                                      in_=st)

            k_proj_window(0)
            nc.gpsimd.collective_compute(
                "AllGather", mybir.AluOpType.bypass, replica_groups=PAIRS,
                ins=[klocs[0][:]], outs=[kgs[0][:]])
            k_proj_window(1)
            nc.gpsimd.collective_compute(
                "AllGather", mybir.AluOpType.bypass, replica_groups=PAIRS,
                ins=[klocs[1][:]], outs=[kgs[1][:]])

            # V of own token half (bf16 out), reusing the xtk windows.
            for w in range(2):
                xtk = xtks[w]
                for ts in range(4):
                    for n in range(2):
                        ps = p512.tile([P, 512], F32, tag="mm512", name="ps_v")
                        for c in range(DC):
                            mm(ps, xtk[:, c * 512 + P * ts: c * 512 + P * (ts + 1)],
                               wv_sb[:, c * D + 512 * n: c * D + 512 * (n + 1)],
                               c == 0, c == DC - 1)
                        st = stg_pool.tile([P, 512], BF16, tag="stv", name="stv")
                        nc.scalar.copy(out=st, in_=ps)
                        nc.sync.dma_start(
                            out=vloc[512 * w + 128 * ts: 512 * w + 128 * (ts + 1),
                                     512 * n: 512 * (n + 1)],
                            in_=st)
            nc.gpsimd.collective_compute(
                "AllGather", mybir.AluOpType.bypass, replica_groups=PAIRS,
                ins=[vloc[:]], outs=[vg[:]])

            # wq + masks ride scalar behind the wv odds (needed only by the
            # Q projection / scores phases)
            wq_sb = wkv_pool.tile([P, DC * D], BF16, tag="wq", name="wq_sb")
            for c in range(DC):
                nc.scalar.dma_start(out=wq_sb[:, c * D:(c + 1) * D],
                                    in_=wq_d[c * P:(c + 1) * P, :])
            for u in range(4):
                nc.scalar.dma_start(out=mask_sb[:, u * QB:(u + 1) * QB],
                                    in_=masks_d[u])
            nc.vector.memset(ones_f32, 1.0)

            # K unpack rides sync, emitted BEFORE the Q projection: its
            # triggers block the sync stream on the K gathers (fine — all
            # later sync work belongs to phases that need K anyway) but
            # crucially NOT the scalar stream, whose Q_sb copies must flow
            # during the Q projection.  Gather slot r of window w holds
            # global tokens [1024r + 512w, 1024r + 512w + 512).
            for w in range(2):
                for r in range(2):
                    gw = 2 * r + w
                    for c in range(DC):
                        nc.sync.dma_start(
                            out=K_sb[:, c // 2, c % 2,
                                     512 * gw:512 * (gw + 1)],
                            in_=kgs[w][r, c * P:(c + 1) * P, :])

            # Q^T projection (own queries, two q-blocks per matmul), fp8 out
            with nc.named_scope("q_proj"):
                for jp in range(NJB // 2):
                    xtq = xtqs[jp]
                    for c2 in range(DC):
                        ps = p512.tile([P, 512], F32, tag="mm512", name="ps_q")
                        for c in range(DC):
                            mm(ps,
                               wq_sb[:, c * D + P * c2: c * D + P * (c2 + 1)],
                               xtq[:, c * 512:(c + 1) * 512], c == 0,
                               c == DC - 1)
                        nc.scalar.copy(
                            out=Q_sb[:, c2 // 2, c2 % 2,
                                     512 * jp:512 * (jp + 1)],
                            in_=ps)
            # NOTE: the V unpack is NOT emitted here.  A dma trigger's
            # semaphore wait blocks its whole engine stream (in-order), so
            # V-unpack triggers emitted before the scores phase would gate
            # the scores on the V AllGather.  _emit_v_unpack() is called
            # between the scores and ctx emissions instead.

            def _emit_v_unpack():
                # 3-way split in consumption order; gpsimd is safe here
                # (its post-CC-trigger wait is on the V gather, which this
                # unpack needs anyway) and shortens the unpack from ~24us
                # to ~16us so the context phase never outruns it.
                engs = [nc.sync, nc.scalar, nc.gpsimd]
                for r in range(2):
                    for tt in range(8):
                        t = 8 * r + tt
                        engs[t % 3].dma_start(
                            out=V_sb[:, t * D:(t + 1) * D],
                            in_=vg[r, 128 * tt:128 * (tt + 1), :])

        # ---- attention: all scores (fp8 DoubleRow), then all contexts ----
        with nc.named_scope("attn"):
            expS = attnp.tile([P, NKT, QB], BF16, tag="E", name="expS")
            # Denominators: pairwise bf16 adds of expS tiles (2x DVE rate)
            # then an fp32 chain over the pairs, interleaved with the score
            # tiles so the DVE overlaps the PE; each q-block's tiny den
            # matmul is emitted one block LATE (jb3's after ctx jb0) so the
            # PE stream never waits on the DVE chain.
            accs, recips = [], []

            def emit_den(jb):
                den = pden.tile([P, 2], F32, tag="den", name="den")
                for s in range(2):
                    nc.tensor.matmul(den[:, s:s + 1],
                                     accs[jb][:, P * s:P * (s + 1)], ones_f32,
                                     start=True, stop=True,
                                     skip_group_check=True)
                recip = recip_pool.tile([P, 2], F32, tag="recip", name="recip")
                nc.vector.reciprocal(recip, den)
                recips.append(recip)

            def emit_ctx(jb):
                kt = 4 * (jb + 1)
                for s in range(2):
                    for n in range(2):
                        # the last 2 k-tiles are fully masked for the s=0
                        # q-sub (their expS is exactly 0) — skip them
                        nt_s = kt - 2 if s == 0 else kt
                        ps = p512.tile([P, 512], F32, tag="mm512",
                                       name="ps_c")
                        for t in range(nt_s):
                            mm(ps, expS[:, OFF[jb] + t, P * s:P * (s + 1)],
                               V_sb[:, t * D + 512 * n: t * D + 512 * (n + 1)],
                               t == 0, t == nt_s - 1)
                        ot = out_pool.tile([P, 512], BF16, tag="out",
                                           name="ot")
                        nc.vector.tensor_scalar_mul(
                            ot, ps, recips[jb][:, s:s + 1])
                        nc.scalar.dma_start(
                            out=out_d[QB * jb + P * s: QB * jb + P * (s + 1),
                                      512 * n: 512 * (n + 1)],
                            in_=ot)

            with nc.named_scope("scores"):
                for jb in range(NJB):
                    kt = 4 * (jb + 1)  # k-tiles needed by this q-block
                    acc = acc_pool.tile([P, QB], F32, tag="acc", name="acc",
                                        bufs=4)
                    accs.append(acc)
                    pairs = []
                    for t in range(kt):
                        ps = p256.tile([P, QB], F32, tag="mm256", name="ps_s")
                        for g in range(DC // 2):
                            nc.tensor.matmul(
                                ps, K_sb[:, g, :, P * t:P * (t + 1)],
                                Q_sb[:, g, :, QB * jb:QB * (jb + 1)],
                                start=(g == 0), stop=(g == DC // 2 - 1),
                                perf_mode=DR)
                        et = expS[:, OFF[jb] + t, :]
                        nc.scalar.activation(out=et, in_=ps, func=_EXP,
                                             scale=SCALE)
                        if t >= kt - 4:
                            u = t - (kt - 4)
                            nc.vector.tensor_mul(
                                et, et, mask_sb[:, u * QB:(u + 1) * QB])
                        if t % 2 == 1:
                            pair = pair_pool.tile([P, QB], BF16, tag="pair",
                                                  name="pair")
                            nc.vector.tensor_add(
                                pair, expS[:, OFF[jb] + t - 1, :], et)
                            pairs.append(pair)
                            if len(pairs) == 1:
                                nc.vector.tensor_copy(acc, pair)
                            else:
                                nc.vector.tensor_add(acc, acc, pair)
                    if jb >= 1:
                        emit_den(jb - 1)
            _emit_v_unpack()
            with nc.named_scope("ctx"):
                emit_ctx(0)
                emit_den(3)
                for jb in range(1, NJB):
                    emit_ctx(jb)


def build_nc():
    nc = bacc.Bacc("TRN2", target_bir_lowering=False, debug=False, num_devices=8)
    xTk_d = nc.dram_tensor("xTk", [D, T // 2], BF16, kind="ExternalInput")
    xTq_d = nc.dram_tensor("xTq", [D, T // 2], BF16, kind="ExternalInput")
    wq_d = nc.dram_tensor("wq", [D, D], BF16, kind="ExternalInput")
    wk_d = nc.dram_tensor("wk", [D, D], BF16, kind="ExternalInput")
    wv_d = nc.dram_tensor("wv", [D, D], BF16, kind="ExternalInput")
    masks_d = nc.dram_tensor("masks", [4, P, QB], BF16, kind="ExternalInput")
    out_d = nc.dram_tensor("out", [T // 2, D], BF16, kind="ExternalOutput")
    with tile.TileContext(nc) as tc:
        _emit(nc, tc, xTk_d[:], xTq_d[:], wq_d[:], wk_d[:], wv_d[:],
              masks_d[:], out_d[:])
    nc.compile()
    return nc


def make_masks(h):
    """Multiplicative causal mask: 1 where key (128u + p) <= query (2j + h),
    else 0, within a 512-position diagonal window (positions relative to
    the q-block base).  Applied to expS after the exp."""
    u = np.arange(4)[:, None, None]
    p = np.arange(P)[None, :, None]
    j = np.arange(QB)[None, None, :]
    vis = (128 * u + p <= 2 * j + h)
    return np.where(vis, 1.0, 0.0).astype(BF16_NP)


def make_in_maps(x, W_query, W_key, W_value):
    wq = np.ascontiguousarray(W_query).astype(BF16_NP)
    wk = np.ascontiguousarray(W_key).astype(BF16_NP)
    wv = np.ascontiguousarray(W_value).astype(BF16_NP)
    masks = [make_masks(h) for h in range(2)]
    in_maps = []
    for core in range(8):
        b, h = divmod(core, 2)
        xb = np.asarray(x[b], dtype=np.float32)
        in_maps.append({
            "xTk": np.ascontiguousarray(xb[1024 * h:1024 * (h + 1)].T)
                   .astype(BF16_NP),
            "xTq": np.ascontiguousarray(xb[h::2].T).astype(BF16_NP),
            "wq": wq, "wk": wk, "wv": wv,
            "masks": masks[h],
        })
    return in_maps


_NC_CACHE = {}
LAST_EXEC_NS = None


def kernel(x, W_query, W_key, W_value):
    global LAST_EXEC_NS
    from concourse.bass_utils import run_bass_kernel_spmd

    if "nc" not in _NC_CACHE:
        _NC_CACHE["nc"] = build_nc()
    nc = _NC_CACHE["nc"]

    in_maps = make_in_maps(x, W_query, W_key, W_value)
    trace = bool(os.environ.get("BASS_TRACE"))
    res = run_bass_kernel_spmd(nc, in_maps, core_ids=list(range(8)), trace=trace)
    LAST_EXEC_NS = res.exec_time_ns

    out = np.empty((B, T, D), dtype=np.float32)
    for core in range(8):
        b, h = divmod(core, 2)
        out[b, h::2, :] = res.results[core]["out"].astype(np.float32)
    return out


if __name__ == "__main__":
    import time
    t0 = time.time()
    nc = build_nc()
    print(f"build+compile took {time.time() - t0:.1f}s")
    print("built ok")
